# revision 9
# baseline (speedup 1.0000x reference)
"""Trainium2 Bass kernel for DomainAdaptationLayer.

Computes, for x (B=131072, D=512) f32:
  normalized_x = per-row LayerNorm(x) with per-subject affine (gamma/beta
                 table rows selected by `groups`; rows are all-ones/zeros
                 for the module's parameters, so the device kernel computes
                 the plain LayerNorm and a host-side fixup applies the
                 affine only if the tables are non-trivial)
  domain_logits = gelu(gelu(x@W1+b1)@W2+b2)@W3+b3   (exact-erf gelu)

Sharding: pure data parallel, B split across 8 NeuronCores, weights
replicated. No cross-device communication.

Per-core dataflow (Bs=16384 rows, 128 tiles of 128 rows, 2-tile blocks):
  DMA   : x tile in (fp32), xn tile out, logits block out
  POOL  : cast x->bf16 with accumulated row-sum (tensor_scalar accum)
  ACT   : x^2 with accumulated row-sum-of-squares; gelu1; gelu2; sqrt
  PE    : transpose x_bf16 -> xT; W1/W2 matmuls in transposed layout
          (h1T, h2T); final matmul uses g2T as stationary so logits come
          out in natural row layout
  DVE   : xT PSUM->SBUF copy; normalize (x*rstd - mu*rstd) via
          tensor_scalar with per-row scalar APs; logits bias add;
          batched per-group stats math (G=16 rows of 128)
"""

import os
import sys

for _p in ("/opt/trn_rl_repo",):
    if _p not in sys.path and os.path.isdir(_p):
        sys.path.insert(0, _p)

import numpy as np

import concourse.bacc as bacc
import concourse.bass as bass
import concourse.mybir as mybir
import concourse.tile as tile
from concourse import bass_utils

F32 = mybir.dt.float32
F16 = mybir.dt.float16        # MLP compute dtype (PE-rate identical, ~8x
NP_F16 = np.float16           # lower rounding error than bf16 for this data

N_CORES = 8
B_FULL, D = 131072, 512
BS = B_FULL // N_CORES        # rows per core
P = 128                       # partitions / rows per tile
H1, H2, NS = 256, 128, 16
EPS = 1e-5

# tuning knobs
GROUP = 16                    # tiles per batched-stats group
BLK = 2                       # row-tiles per MLP block (moving free = 256)
CAST_ENGINE = "dve"           # engine for the cast+rowsum pass


def build_nc(n_rows=BS, gelu_fn=None, cast_engine=CAST_ENGINE):
    """Build the per-core Bass module. n_rows must be a multiple of
    GROUP*P for the main loop (smaller multiples of BLK*P allowed for
    sim testing; the group then shrinks)."""
    if gelu_fn is None:
        gelu_fn = mybir.ActivationFunctionType.Gelu
    n_tiles = n_rows // P
    assert n_rows % (BLK * P) == 0
    group = min(GROUP, n_tiles)
    assert n_tiles % group == 0 and group % BLK == 0

    nc = bacc.Bacc("TRN2", target_bir_lowering=False, debug=False)
    x_d = nc.declare_dram_parameter("x", [n_rows, D], F32, isOutput=False)
    w1_d = nc.declare_dram_parameter("w1", [D, H1], F16, isOutput=False)
    w2_d = nc.declare_dram_parameter("w2", [H1, H2], F16, isOutput=False)
    w3_d = nc.declare_dram_parameter("w3", [H2, NS], F16, isOutput=False)
    b1_d = nc.declare_dram_parameter("b1", [H1], F32, isOutput=False)
    b2_d = nc.declare_dram_parameter("b2", [H2], F32, isOutput=False)
    id_d = nc.declare_dram_parameter("ident", [P, P], F16, isOutput=False)
    eps_d = nc.declare_dram_parameter("epsc", [P, 1], F32, isOutput=False)
    xn_d = nc.declare_dram_parameter("xn", [n_rows, D], F32, isOutput=True)
    lg_d = nc.declare_dram_parameter("logits", [n_rows, NS], F32, isOutput=True)

    mult = mybir.AluOpType.mult
    add = mybir.AluOpType.add
    subtract = mybir.AluOpType.subtract

    with tile.TileContext(nc) as tc:
        with (
            tc.tile_pool(name="consts", bufs=1) as pc,
            tc.tile_pool(name="xf", bufs=group + 4) as pxf,
            tc.tile_pool(name="xb", bufs=8) as pxb,
            tc.tile_pool(name="sqs", bufs=3) as psq,
            tc.tile_pool(name="xt", bufs=2) as pxt,
            tc.tile_pool(name="g1", bufs=2) as pg1,
            tc.tile_pool(name="g2", bufs=2) as pg2,
            tc.tile_pool(name="lg", bufs=3) as plg,
            tc.tile_pool(name="xn", bufs=6) as pxn,
            tc.tile_pool(name="strips", bufs=8) as pstr,
            tc.tile_pool(name="stats", bufs=4) as pst,
            tc.tile_pool(name="ps_t", bufs=2, space="PSUM") as ptp,
            tc.tile_pool(name="ps_h1", bufs=2, space="PSUM") as ph1,
            tc.tile_pool(name="ps_h2", bufs=2, space="PSUM") as ph2,
            tc.tile_pool(name="ps_lg", bufs=2, space="PSUM") as plgp,
        ):
            # ---- constants / weights ----
            w1sb = pc.tile([P, D // P, H1], F16)     # [p, kc, j]
            nc.sync.dma_start(w1sb[:], w1_d[:].rearrange("(c p) j -> p c j", p=P))
            w2sb = pc.tile([P, H1 // P, H2], F16)
            nc.sync.dma_start(w2sb[:], w2_d[:].rearrange("(c p) j -> p c j", p=P))
            w3sb = pc.tile([P, NS], F16)
            nc.sync.dma_start(w3sb[:], w3_d[:])
            b1sb = pc.tile([P, H1 // P], F32)
            nc.sync.dma_start(b1sb[:], b1_d[:].rearrange("(c p) -> p c", p=P))
            b2sb = pc.tile([P, 1], F32)
            nc.sync.dma_start(b2sb[:], b2_d[:].rearrange("(c p) -> p c", p=P))
            epssb = pc.tile([P, 1], F32)
            nc.sync.dma_start(epssb[:], eps_d[:])
            ident = pc.tile([P, P], F16)
            nc.sync.dma_start(ident[:], id_d[:])

            n_groups = n_tiles // group
            blocks_per_group = group // BLK
            for g in range(n_groups):
                ssum = pstr.tile([P, group], F32, tag="ssum")
                ssq = pstr.tile([P, group], F32, tag="ssq")
                xfs = []
                for blk in range(blocks_per_group):
                    t0 = g * group + blk * BLK    # first tile of block
                    xbs = []
                    for i in range(BLK):
                        t = t0 + i
                        gi = blk * BLK + i        # index within group
                        xf = pxf.tile([P, D], F32, tag="xf")
                        nc.sync.dma_start(xf[:], x_d[bass.ts(t, P), :])
                        xfs.append(xf)
                        # cast to bf16 + row sums
                        xb = pxb.tile([P, D], F16, tag="xb")
                        cast_eng = (nc.gpsimd if cast_engine == "pool"
                                    else nc.vector)
                        cast_eng.tensor_scalar(
                            out=xb[:], in0=xf[:], scalar1=1.0, scalar2=None,
                            op0=mult, op1=add,
                            accum_out=ssum[:, gi:gi + 1])
                        xbs.append(xb)
                        # x^2 + row sums of squares
                        sqf = psq.tile([P, D], F16, tag="sqs")
                        nc.scalar.activation(
                            sqf[:], xf[:], mybir.ActivationFunctionType.Square,
                            accum_out=ssq[:, gi:gi + 1])

                    # transpose block to [d, r] layout (bf16)
                    xtp = ptp.tile([P, D // P, BLK * P], F16, tag="ps_t")
                    for i in range(BLK):
                        for c in range(D // P):
                            nc.tensor.transpose(
                                xtp[:, c, bass.ts(i, P)],
                                xbs[i][:, bass.ts(c, P)], ident[:])
                    xt = pxt.tile([P, D // P, BLK * P], F16, tag="xt")
                    nc.vector.tensor_copy(xt[:], xtp[:])

                    # MLP layer 1: h1T[j, r] += W1[k, j]^T x xT[k, r]
                    h1p = ph1.tile([P, H1 // P, BLK * P], F32, tag="ps_h1")
                    for j in range(H1 // P):
                        for c in range(D // P):
                            nc.tensor.matmul(
                                h1p[:, j, :],
                                w1sb[:, c, bass.ts(j, P)],
                                xt[:, c, :],
                                start=(c == 0), stop=(c == D // P - 1))
                    g1 = pg1.tile([P, H1 // P, BLK * P], F16, tag="g1")
                    for j in range(H1 // P):
                        nc.scalar.activation(
                            g1[:, j, :], h1p[:, j, :], gelu_fn,
                            bias=b1sb[:, j:j + 1])

                    # MLP layer 2
                    h2p = ph2.tile([P, BLK * P], F32, tag="ps_h2")
                    for c in range(H1 // P):
                        nc.tensor.matmul(
                            h2p[:], w2sb[:, c, :], g1[:, c, :],
                            start=(c == 0), stop=(c == H1 // P - 1))
                    g2 = pg2.tile([P, BLK * P], F16, tag="g2")
                    nc.scalar.activation(g2[:], h2p[:], gelu_fn,
                                         bias=b2sb[:, 0:1])

                    # MLP layer 3: natural layout via g2T as stationary
                    lgp = plgp.tile([P, BLK, NS], F32, tag="ps_lg")
                    lg = plg.tile([P, BLK, NS], F32, tag="lg")
                    for i in range(BLK):
                        nc.tensor.matmul(
                            lgp[:, i, :], g2[:, bass.ts(i, P)], w3sb[:],
                            start=True, stop=True)
                    nc.scalar.copy(lg[:], lgp[:])
                    nc.sync.dma_start(
                        lg_d[bass.ts(t0 // BLK, BLK * P), :]
                        .rearrange("(t p) j -> p t j", p=P),
                        lg[:])

                # ---- batched stats for the group ----
                nmu = pst.tile([P, group], F32, tag="nmu")
                nc.vector.tensor_scalar(out=nmu[:], in0=ssum[:],
                                        scalar1=-1.0 / D, scalar2=None,
                                        op0=mult)
                msq = pst.tile([P, group], F32, tag="msq")
                nc.vector.tensor_scalar(out=msq[:], in0=ssq[:],
                                        scalar1=1.0 / D, scalar2=None,
                                        op0=mult)
                var = pst.tile([P, group], F32, tag="var")
                nc.vector.tensor_tensor(out=var[:], in0=nmu[:], in1=nmu[:],
                                        op=mult)
                nc.vector.tensor_tensor(out=var[:], in0=msq[:], in1=var[:],
                                        op=subtract)
                std = pst.tile([P, group], F32, tag="std")
                nc.scalar.activation(std[:], var[:],
                                     mybir.ActivationFunctionType.Sqrt,
                                     bias=epssb[:, 0:1])
                rstd = pst.tile([P, group], F32, tag="rstd")
                nc.vector.reciprocal(rstd[:], std[:])
                nmr = pst.tile([P, group], F32, tag="nmr")
                nc.vector.tensor_tensor(out=nmr[:], in0=nmu[:], in1=rstd[:],
                                        op=mult)

                # ---- normalize + store the group's tiles ----
                for gi in range(group):
                    t = g * group + gi
                    xn = pxn.tile([P, D], F32, tag="xn")
                    nc.vector.tensor_scalar(
                        out=xn[:], in0=xfs[gi][:],
                        scalar1=rstd[:, gi:gi + 1],
                        scalar2=nmr[:, gi:gi + 1],
                        op0=mult, op1=add)
                    nc.sync.dma_start(xn_d[bass.ts(t, P), :], xn[:])

    nc.finalize()
    return nc


_NC_CACHE = {}


def _get_nc():
    key = (BS, CAST_ENGINE)
    if key not in _NC_CACHE:
        _NC_CACHE[key] = build_nc()
    return _NC_CACHE[key]


def kernel(x, groups, gammas, betas, W1, b1, W2, b2, W3, b3):
    x = np.ascontiguousarray(np.asarray(x, dtype=np.float32))
    W1 = np.asarray(W1, dtype=np.float32)
    W2 = np.asarray(W2, dtype=np.float32)
    W3 = np.asarray(W3, dtype=np.float32)
    b1 = np.ascontiguousarray(np.asarray(b1, dtype=np.float32))
    b2 = np.ascontiguousarray(np.asarray(b2, dtype=np.float32))
    b3 = np.ascontiguousarray(np.asarray(b3, dtype=np.float32))

    nc = _get_nc()
    ident_np = np.eye(P, dtype=NP_F16)
    eps_np = np.full((P, 1), EPS, dtype=np.float32)
    w1b = np.ascontiguousarray(W1.astype(NP_F16))
    w2b = np.ascontiguousarray(W2.astype(NP_F16))
    w3b = np.ascontiguousarray(W3.astype(NP_F16))
    in_maps = []
    for c in range(N_CORES):
        in_maps.append({
            "x": x[c * BS:(c + 1) * BS],
            "w1": w1b, "w2": w2b, "w3": w3b,
            "b1": b1, "b2": b2,
            "ident": ident_np, "epsc": eps_np,
        })
    kernel.last_in_maps = in_maps
    res = bass_utils.run_bass_kernel_spmd(
        nc, in_maps, list(range(N_CORES)),
        trace=bool(int(os.environ.get("KERNEL_TRACE", "0"))),
    )
    kernel.last_results = res
    xn = np.concatenate([res.results[c]["xn"] for c in range(N_CORES)], axis=0)
    logits = np.concatenate([res.results[c]["logits"] for c in range(N_CORES)],
                            axis=0)
    logits = logits + b3[None, :]

    gammas = np.asarray(gammas, dtype=np.float32)
    betas = np.asarray(betas, dtype=np.float32)
    if not (np.all(gammas == 1.0) and np.all(betas == 0.0)):
        # Non-trivial affine tables: apply the per-subject affine on host.
        g = np.asarray(groups)
        valid = (g >= 0) & (g < gammas.shape[0] - 1)
        idx = np.where(valid, g, gammas.shape[0] - 1).astype(np.int64)
        xn = xn * gammas[idx] + betas[idx]
    return xn, logits
